# revision 29
# baseline (speedup 1.0000x reference)
"""Trainium2 Bass kernel for nn_CompletenessLoss (OHEM hinge loss with top-k).

Self-contained: accepts FULL inputs, shards over 8 NeuronCores internally
(data-parallel over the group dimension), returns the full scalar output.

Math (reference):
  scores[i]  = pred[i, labels[i]-1]
  groups of 64 rows: first 8 are "positive", last 56 are "negative"
  pos_ls = sum over all positive rows of relu(1 - s)
  neg_ls = sum over groups of (sum of top-9 of relu(1 + s) over 56 negatives)
  out    = (pos_ls + neg_ls) / (num_pos + int(num_neg * 0.17))

V2 gather strategy (per core, 32768 rows = 128 partitions x 256 rows):
  The label-indexed gather is split across two engines running in parallel:
  - rows t in [0, XG): GPSIMD ap_gather. Indices are shared per 16-partition
    group, so the host packs index lists where position 16k+q holds the
    index of partition (16g+q)'s row k; each partition's wanted value lands
    on the "diagonal" position 16k+(p%16). A static-per-input mask multiply
    + segmented reduce (DVE, cheap) extracts the diagonal.
  - rows t in [XG, 256): DVE scalar_tensor_tensor (iota==label)*pred with
    fused accumulate (the V1 path).
  Both write into one scores[P, 256] tile; phase 2 (hinge + top-9) as in V1.

  Measured constraints that pin this design (from session traces):
  - ap_gather costs ~28.5ns/index (RD_CMD serialization, ReadOverlap=0),
    independent of instruction granularity -> the gather lane saturates at
    ~80-100 rows within the DMA window.
  - DVE mask-select costs ~270ns/row incl. the one-hot mask DMA (400B/row),
    and GPSIMD shares its SBUF port with DVE (gathers run ~2x slower while
    DVE is busy), so the kernel is jointly DMA- and DVE-bound at ~98us.
"""

import numpy as np

# Problem geometry (hardcoded per the harness contract).
N_FULL = 262144
D = 200                      # pred_dim
GS = 64                      # sample_group_size
SS = 8                       # sample_split (positives per group)
OHEM_RATIO = 0.17
KEEP = int((GS - SS) * OHEM_RATIO)   # 9 hardest negatives kept per group

N_CORES = 8
ROWS = N_FULL // N_CORES     # 32768 rows per core
P = 128                      # SBUF partitions
NTILES = ROWS // P           # 256 rows per partition = 4 groups
CHUNK = 16                   # rows-per-partition per DMA/compute chunk
XG = 80                      # rows per partition gathered on GPSIMD
XS = NTILES - XG             # rows via host one-hot mask + DVE mult/reduce

_compiled = None             # cached program so repeat calls skip rebuild


def build_nc():
    """Build the per-core Bass program (SPMD across the 8 cores)."""
    import concourse.bacc as bacc
    import concourse.tile as tile
    from concourse import mybir

    f32 = mybir.dt.float32
    bf16 = mybir.dt.bfloat16
    i16 = mybir.dt.int16

    nc = bacc.Bacc("TRN2", target_bir_lowering=False, debug=False,
                   num_devices=N_CORES)
    pred_t = nc.dram_tensor("pred", [ROWS, D], bf16, kind="ExternalInput")
    # idx[p, t] = (t%16)*100 + lab>>1 for gpsimd rows t in [0, XG)
    idx_t = nc.dram_tensor("idx", [P, XG], i16, kind="ExternalInput")
    # msk[p, t*32 + q*2 + e] = (q == p%16) & (e == lab%2), bf16
    msk_t = nc.dram_tensor("msk", [P, XG * 32], bf16, kind="ExternalInput")
    # smask: host one-hot mask for the s-lane rows, streamed per chunk
    smask_t = nc.dram_tensor("smask", [P, XS * D], bf16, kind="ExternalInput")
    out_t = nc.dram_tensor("partial", [P, 2], f32, kind="ExternalOutput")

    with tile.TileContext(nc) as tc:
        _body(tc, pred_t.ap(), idx_t.ap(), msk_t.ap(), smask_t.ap(),
              out_t.ap())
    nc.compile()
    return nc


def _body(tc, pred, idx, msk, smask, out):
    from concourse import mybir
    import concourse.bass as bass
    from contextlib import ExitStack

    nc = tc.nc
    f32 = mybir.dt.float32
    bf16 = mybir.dt.bfloat16
    i16 = mybir.dt.int16
    AX = mybir.AxisListType
    OP = mybir.AluOpType
    AF = mybir.ActivationFunctionType

    with ExitStack() as ctx:
        singles = ctx.enter_context(tc.tile_pool(name="singles", bufs=1))
        ph2 = ctx.enter_context(tc.tile_pool(name="ph2", bufs=2))
        scr = ctx.enter_context(tc.tile_pool(name="scr", bufs=3))

        # --- warm-up FIRST, with zero DMA dependencies, so the gather
        # ucode IRAM load (~6us) and engine dispatch are paid by ~15us.
        wz_idx = singles.tile([P, 16], i16)
        nc.gpsimd.memset(wz_idx, 0)
        wdat = singles.tile([P, 16, 2], bf16)
        nc.gpsimd.memset(wdat.rearrange("p a b -> p (a b)"), 0)
        warm3 = singles.tile([P, 16, 2], bf16)
        nc.gpsimd.ap_gather(out_ap=warm3, in_ap=wdat,
                            idxs_ap=wz_idx[:, 0:1],
                            channels=P, num_elems=16, d=2, num_idxs=16)
        wv = singles.tile([P, 2], f32)
        nc.vector.memset(wv, 0.0)
        nc.vector.tensor_scalar(out=wv, in0=wv, scalar1=0.0, scalar2=1.0,
                                op0=OP.mult, op1=OP.mult)
        wa = singles.tile([P, 2], f32)
        nc.scalar.activation(out=wa, in_=wv, func=AF.Relu,
                             bias=1.0, scale=-1.0)

        # --- one-time inputs: idx first (gathers need it early); the
        # extract masks (msks) are DMA'd after the first two smask chunks
        # so lane-B's first chunk lands as early as possible.
        idxs = singles.tile([P, XG], i16)
        nc.scalar.dma_start(out=idxs, in_=idx)
        msks = singles.tile([P, XG, 32], bf16)

        pred_sb = singles.tile([P, NTILES, D], bf16)
        out2 = singles.tile([P, XG * 16, 2], bf16)
        scores = singles.tile([P, NTILES], f32)

        # --- phase 1: stream pred; gather on GPSIMD + DVE in parallel ---
        pred_v = pred.rearrange("(p t) j -> p t j", p=P)
        # gather chunks grow geometrically (amortize ~2.4us/op overhead);
        # DVE chunks stay small for pipelining. DMA order feeds both early.
        gchunks = [(i * CHUNK, CHUNK) for i in range(XG // CHUNK)]
        # s-lane uses 32-row chunks: 1.64MB DMAs sustain a higher rate
        # than 0.82MB ones and halve the DVE op count for the same bytes.
        SCH = 48
        schunks = [(XG + i * SCH, SCH) for i in range(XS // SCH)]
        if XS % SCH:
            schunks.append((XG + (XS // SCH) * SCH, XS % SCH))
        # Single-queue FIFO: cross-queue packet interleave measured only
        # ~261GB/s (HBM locality thrash) vs ~390GB/s single-queue.  Order
        # front-loads gather chunks so the contention-paced gather chain
        # (~10-13us/chunk) is never data-starved.
        order = [("g", gchunks[0]), ("g", gchunks[1]), ("s", schunks[0]),
                 ("g", gchunks[2]), ("s", schunks[1]),
                 ("g", gchunks[3]), ("s", schunks[2]),
                 ("g", gchunks[4])] + [("s", c) for c in schunks[3:]]

        n_s_issued = 0
        for kind, (tb, csz) in order:
            nc.sync.dma_start(out=pred_sb[:, tb:tb + csz, :],
                              in_=pred_v[:, tb:tb + csz, :])
            if kind == "s":
                n_s_issued += 1
                if n_s_issued == 3:
                    nc.scalar.dma_start(out=msks, in_=msk)
            if kind == "g":
                # gpsimd gather: shared indices per 16-partition group
                nc.gpsimd.ap_gather(
                    out_ap=out2[:, tb * 16:(tb + csz) * 16, :],
                    in_ap=pred_sb[:, tb:tb + csz, :].rearrange(
                        "p t (a b) -> p (t a) b", b=2),
                    idxs_ap=idxs[:, tb:tb + csz],
                    channels=P, num_elems=csz * (D // 2), d=2,
                    num_idxs=csz * 16)
            elif kind == "s":
                # one-hot mask chunk rides the same sync queue right
                # behind its pred chunk; DVE does mult + folds + reduce
                mk = scr.tile([P, 48, D], bf16, tag="mk")
                sm_v = smask.rearrange("p (t j) -> p t j", j=D)
                nc.scalar.dma_start(out=mk[:, 0:csz, :],
                                    in_=sm_v[:, tb - XG:tb - XG + csz, :])
                pr = pred_sb[:, tb:tb + csz, :]
                nc.vector.tensor_tensor(out=pr, in0=pr, in1=mk[:, 0:csz, :],
                                        op=OP.mult)
                # masked rows are one-nonzero-among-zeros: bf16 pairwise
                # fold is exact and runs at 2x; the 1x reduce sees half
                nc.vector.tensor_tensor(
                    out=pr[:, :, 0:D // 2], in0=pr[:, :, 0:D // 2],
                    in1=pr[:, :, D // 2:D], op=OP.add)
                nc.vector.tensor_tensor(
                    out=pr[:, :, 0:D // 4], in0=pr[:, :, 0:D // 4],
                    in1=pr[:, :, D // 4:D // 2], op=OP.add)
                nc.vector.tensor_tensor(
                    out=pr[:, :, 0:D // 8], in0=pr[:, :, 0:D // 8],
                    in1=pr[:, :, D // 8:D // 4], op=OP.add)
                nc.vector.tensor_reduce(
                    out=scores[:, tb:tb + csz], in_=pr[:, :, 0:D // 8],
                    axis=AX.X, op=OP.add)

        # extracts AFTER all stt issues: the Vector queue is in-order, so an
        # extract waiting on a late gather must not block pending stt work.
        for tb, csz in gchunks:
            o2 = out2[:, tb * 16:(tb + csz) * 16, :]
            nc.vector.tensor_tensor(
                out=o2, in0=o2,
                in1=msks[:, tb:tb + csz, :].rearrange(
                    "p t (a b) -> p (t a) b", b=2),
                op=OP.mult)
            o3 = out2[:, tb * 16:(tb + csz) * 16, :].rearrange(
                "p (t a) b -> p t (a b)", a=16)
            nc.vector.tensor_tensor(
                out=o3[:, :, 0:16], in0=o3[:, :, 0:16], in1=o3[:, :, 16:32],
                op=OP.add)
            nc.vector.tensor_reduce(
                out=scores[:, tb:tb + csz], in_=o3[:, :, 0:16],
                axis=AX.X, op=OP.add)

        # --- phase 2: per partition, 4 whole groups along the free axis ---
        gpp = NTILES // GS
        pp = singles.tile([P, gpp], f32)             # pos sums per group
        negacc = singles.tile([P, 2 * gpp], f32)     # top8-sum & 9th cols
        for g in range(gpp):
            stg = scores[:, g * GS:(g + 1) * GS]
            ptmp = ph2.tile([P, SS], f32, tag="ptmp")
            nc.scalar.activation(
                out=ptmp, in_=stg[:, 0:SS], func=AF.Relu,
                bias=1.0, scale=-1.0, accum_out=pp[:, g:g + 1])
            nl = ph2.tile([P, GS - SS], f32, tag="nl")
            nc.scalar.activation(
                out=nl, in_=stg[:, SS:GS],
                func=AF.Relu, bias=1.0, scale=1.0)
            m8 = ph2.tile([P, 8], f32, tag="m8")
            nc.vector.max(out=m8, in_=nl)
            nc.vector.match_replace(
                out=nl, in_to_replace=m8, in_values=nl, imm_value=-1.0)
            s8 = ph2.tile([P, 8], f32, tag="s8")
            nc.scalar.activation(
                out=s8, in_=m8, func=AF.Relu, bias=0.0, scale=1.0,
                accum_out=negacc[:, 2 * g:2 * g + 1])
            nc.vector.tensor_reduce(
                out=negacc[:, 2 * g + 1:2 * g + 2], in_=nl, axis=AX.X,
                op=OP.max)

        # --- final per-partition reduction -> [P, 2] (on Scalar) ---
        res = singles.tile([P, 2], f32)
        fp = ph2.tile([P, gpp], f32, tag="fp")
        nc.scalar.activation(out=fp, in_=pp, func=AF.Relu, bias=0.0,
                             scale=1.0, accum_out=res[:, 0:1])
        fn = ph2.tile([P, 2 * gpp], f32, tag="fn")
        nc.scalar.activation(out=fn, in_=negacc, func=AF.Relu, bias=0.0,
                             scale=1.0, accum_out=res[:, 1:2])
        nc.sync.dma_start(out=out, in_=res)


def _get_compiled():
    global _compiled
    if _compiled is None:
        _compiled = build_nc()
    return _compiled


def _prep_core_inputs(pred, labels):
    """Split full inputs into per-core input maps."""
    import ml_dtypes
    pred = np.asarray(pred).astype(ml_dtypes.bfloat16)
    lab = np.asarray(labels).astype(np.int64)
    k16 = (np.arange(XG, dtype=np.int64) % CHUNK)[None, :]      # [1, XG]
    qsel = (np.arange(P, dtype=np.int64) % 16)                  # [P]
    in_maps = []
    for c in range(N_CORES):
        sl = slice(c * ROWS, (c + 1) * ROWS)
        lab_sh = (lab[sl] - 1).reshape(P, NTILES)                # int64
        lg = lab_sh[:, :XG]                                      # [P, XG]
        idxs = (k16 * (D // 2) + (lg >> 1)).astype(np.int16)
        # msk[p, t, q*2+e] = (q == p%16) & (e == lab%2)
        msk = np.zeros((P, XG, 32), dtype=ml_dtypes.bfloat16)
        e = (lg & 1).astype(np.int64)                            # [P, XG]
        pi = np.arange(P)[:, None]
        ti = np.arange(XG)[None, :]
        msk[pi, ti, qsel[:, None] * 2 + e] = 1
        ls = lab_sh[:, XG:]                                      # [P, XS]
        smask = np.zeros((P, XS, D), dtype=ml_dtypes.bfloat16)
        smask[np.arange(P)[:, None], np.arange(XS)[None, :], ls] = 1
        in_maps.append({
            "pred": np.ascontiguousarray(pred[sl]),
            "smask": np.ascontiguousarray(smask.reshape(P, XS * D)),
            "idx": np.ascontiguousarray(idxs),
            "msk": np.ascontiguousarray(msk.reshape(P, XG * 32)),
        })
    return in_maps


def _finalize(results):
    pos = 0.0
    neg = 0.0
    for r in results:
        part = r["partial"].astype(np.float64)
        pos += part[:, 0].sum()
        neg += part[:, 1].sum()
    num_pos = (N_FULL // GS) * SS
    num_neg = N_FULL - num_pos
    denom = float(num_pos + int(num_neg * OHEM_RATIO))
    return np.float32((pos + neg) / denom)


def kernel(pred, labels, sample_split, sample_group_size):
    assert int(sample_split) == SS and int(sample_group_size) == GS
    from concourse.bass_utils import run_bass_kernel_spmd

    nc = _get_compiled()
    in_maps = _prep_core_inputs(pred, labels)
    res = run_bass_kernel_spmd(nc, in_maps, core_ids=list(range(N_CORES)))
    return _finalize(res.results)
